# revision 71
# baseline (speedup 1.0000x reference)
"""Trainium2 Bass kernel for nn_MetaUpSample (2x meta-upsample, 3x3 dynamic filters).

out[b,ho,wo,f] = sum_k patches[b,ho,wo,k] * meta_w[b,ho,wo,k*3+f]
  patches[b,ho,wo,(dk0,dk1,c)] = x_pad[b, ho//2+dk0, wo//2+dk1, c]

Sharding: 8 cores, core ci handles b = ci//2, ho in [(ci%2)*64, (ci%2)*64+64).
meta_w (432 MiB total) is the dominant HBM stream (~56.6 MiB/core); the kernel
streams it once (partition = wo) and fuses multiply+reduce in single DVE
scalar_tensor_tensor ops: out = (mw * 1.0) * patch, accum_out = per-partition
sum over K. 3 ops per output row (one per filter), 192 per core.

Host side pre-builds (a) the duplicated patch-row tensor xrb (x is only 4 MiB:
xrb[wo, hp, :] = the 192 floats of padded x row hp that pixel column wo needs),
(b) an F-major relayout of meta_w so the strided operand becomes contiguous,
and un-transposes the [wo, (ho,f)] device output. The device graph is just:
  1 xrb DMA (ACT ring) + 16 x (3.54MiB meta_w DMA + 12 fused ops) + 1 out DMA.
"""
from contextlib import ExitStack

import numpy as np
import ml_dtypes

import concourse.bass as bass
import concourse.mybir as mybir
from concourse.bass_utils import run_bass_kernel_spmd

B, H, W, C = 4, 64, 64, 64
HO, WO, F = 128, 128, 3
KS = 3
K = KS * KS * C            # 576
QF = K * F                 # 1728 meta_w channels
RW = KS * C                # 192 floats per patch row (dk1, c)
N_CORES = 8
CORES_PER_B = N_CORES // B         # 2
HO_PC = HO // CORES_PER_B          # 64 output rows per core
NHS = HO_PC // 2                   # 32 hs tiles per core
NROWS = NHS + 2                    # 34 cached padded x rows per core

import os

# Tuned on HW (TRN2, 8 cores). Notes from the sweep:
#  - deeper prefetch (NBUF*RPT >= ~14 rows) consistently REGRESSES (SBUF
#    port/bank contention between the DMA writes and DVE reads);
#  - F-major host layout makes the STT in0 read contiguous: 834 -> 685 ns/op;
#  - mixed small/large tile ramp schedules regress (DMA is the bottleneck
#    engine; small tiles starve it).
NBUF = int(os.environ.get("K_NBUF", "8"))    # meta_w double-buffer slots
RPT = int(os.environ.get("K_RPT", "2"))      # meta_w rows per DMA tile
NSCR = int(os.environ.get("K_NSCR", "2"))    # DVE scratch ring slots
SELF_WAITS = os.environ.get("K_WAITS", "0") == "1"
FMAJOR = os.environ.get("K_FMAJOR", "1") == "1"  # host-transpose mw to [f,k]
# xrb tail re-order: with XTAIL the xrb head covers the first XH x-rows and
# the tail is issued from the SYNC ring after tile XTI, so its ~1.2MB does
# not compete with the first meta_w tiles for early HBM bandwidth.
XTAIL = os.environ.get("K_XTAIL", "0") == "1"
XTI = int(os.environ.get("K_XTI", "3"))
XH = int(os.environ.get("K_XH", "9")) if XTAIL else RPT + 1
# xrb broadcast-load: ship the un-duplicated [64, NROWS*RW] x-row tensor and
# let the DMA replicate each source partition to the two wo partitions that
# need it (stride-0 source dim) -> halves xrb HBM bytes.
XB = os.environ.get("K_XB", "0") == "1"

f32 = mybir.dt.float32
bf16 = mybir.dt.bfloat16
BF16 = os.environ.get("K_BF16", "1") == "1"  # stream mw/x in bf16 (2e-2 tol)
DT = bf16 if BF16 else f32
NPDT = ml_dtypes.bfloat16 if BF16 else np.float32
AB16 = os.environ.get("K_AB16", "0") == "1"  # bf16 accum_out (all-2B STT)
ADT = bf16 if AB16 else f32
NPADT = ml_dtypes.bfloat16 if AB16 else np.float32
GF = int(os.environ.get("K_GF", "0"))  # filters offloaded to GpSimd (0..3)
TTR = os.environ.get("K_TTR", "0") == "1"  # use tensor_tensor_reduce op
# 2-op scheme: one big tensor_tensor mult (row-pair x 3 filters = 6x576
# elems, window broadcast via stride-0 AP) + one segmented tensor_reduce.
# Motivation: plain tensor_tensor/tensor_reduce are eligible for the DVE
# 2x_1P packed 16-bit mode; scalar_tensor_tensor is not.
TT2 = os.environ.get("K_TT2", "0") == "1"
NPROD = int(os.environ.get("K_NPROD", "3"))  # product ring slots (pairs)
PAIR = 2 * F  # 6 segments of K elems per row-pair
# ACT-offload: per row-pair, DVE runs fused STT for the first PAIR-ACTR
# segments and ONE packed tensor_tensor mult (2 elem/cycle in bf16) for the
# last ACTR; the idle ACT engine reduces those products via
# activation(Copy, accum_out).  Probed costs: STT 671ns/seg, packed mult
# 311ns/seg, ACT reduce ~850ns/seg — ACTR=3 balances DVE vs ACT.
ACTR = int(os.environ.get("K_ACTR", "3"))
NPAIRS = HO_PC // 2
# mixed split: MIX4 of the 32 pairs give ACT one extra segment (b=ACTR+1).
# continuous optimum b*~3.27 -> MIX4~9 balances DVE (93.4us) vs ACT (93.4us)
MIX4 = int(os.environ.get("K_MIX4", "10"))
B_OF = [ACTR] * NPAIRS
# place the (DVE-faster) b=ACTR+1 pairs after the DMA-bound startup phase
MIXLO = int(os.environ.get("K_MIXLO", "0"))
if ACTR and MIX4:
    for i in range(MIX4):
        B_OF[
            min(MIXLO + round((i + 0.5) * (NPAIRS - MIXLO) / MIX4), NPAIRS - 1)
        ] = ACTR + 1
MAXB = max(B_OF) if ACTR else 0
CUM_A = [0]
CUM_S = [0]
for _s in range(NPAIRS):
    CUM_A.append(CUM_A[-1] + B_OF[_s])
    CUM_S.append(CUM_S[-1] + (PAIR - B_OF[_s]))
P1 = int(os.environ.get("K_P1", "31"))  # pairs stored in the first out chunk
# Of the ACTR mult segments, GpSimd (own SBUF port pair, tensor_tensor
# supported, ~1.1ns/elem) multiplies the last GM; DVE packed-mults the rest.
GM = int(os.environ.get("K_GM", "0"))
assert GM <= ACTR
# DMA-fold: GpSimd issues ONE SWDGE SBUF->SBUF accumulate-DMA per pair that
# folds each prod segment's upper half into its lower half
# (prod[g][0:K/2] += prod[g][K/2:K] for all ACTR segments), halving the ACT
# reduce stream.  Uses idle SDMA headroom + the idle GpSimd sequencer.
DF = os.environ.get("K_DF", "0") == "1"
assert not (DF and (GM or not ACTR))
# F1: per pair DVE does ONE packed mult over all 6 segments plus ONE packed
# fold (halves added: [6,576]->[6,288]); ACT reduces six 288-elem halves.
# DVE 2889ns/pair (vs 3016 for mult3+3xSTT), ACT 2772ns/pair.
F1 = os.environ.get("K_F1", "0") == "1"
assert not (F1 and (ACTR or TT2))
# head-feed: rows 0-1 of tile 0 arrive via the (otherwise idle-at-start) ACT
# HWDGE ring in parallel with sync's rows 2-3, so DVE's first pair starts
# ~6us earlier.  Own semaphore: arrival order of the two halves is unordered.
# host ships mw as [WO, HO_PC, QF]: the per-tile DMA reads one contiguous
# rows*QF chunk per partition (27.6KB at RPT=4) instead of RPT strided 6.9KB
# chunks -> fewer/bigger descriptors, better HBM efficiency
WOMAJOR = os.environ.get("K_WOMAJOR", "1") == "1"
HF = (os.environ.get("K_HF", "0") == "1") and ACTR > 0 and WOMAJOR
PAIRED = TT2 or ACTR > 0 or F1

if PAIRED:
    # tiles must hold whole row-pairs; small head tiles so compute starts
    # sooner (DMA has headroom in this regime), small tail for a quick drain
    if os.environ.get("K_HEAD", "1") == "1":
        SCHED = [2, 2] + [RPT] * ((HO_PC - 8) // RPT) + [2, 2]
    else:
        SCHED = [RPT] * ((HO_PC - 4) // RPT) + [2, 2]
elif os.environ.get("K_RAMP", "0") == "1":
    SCHED = [1, 1, 2] + [RPT] * ((HO_PC - 8) // RPT) + [2, 1, 1]
elif os.environ.get("K_TAIL", "1") == "1":
    # small TAIL tiles only: after the last meta_w byte lands, DVE has just a
    # 1-row tile (3 ops, ~2us) left instead of a 4-row one (~8us). Small HEAD
    # tiles regress (they starve the DMA stream early), so the head stays 4-row.
    SCHED = [RPT] * ((HO_PC - 4) // RPT) + [2, 1, 1]
else:
    SCHED = [RPT] * (HO_PC // RPT)
assert sum(SCHED) == HO_PC
NT = len(SCHED)
ROW0 = [sum(SCHED[:t]) for t in range(NT)]          # first ho row of tile t
OPS0 = [ROW0[t] * F for t in range(NT)]             # DVE ops before tile t
MAXR = max(SCHED)

_CACHED = None


def _xrb_src(xrb_d, lo, hi):
    """DRAM source AP for xrow[:, lo:hi]; with XB each of the 64 source
    partitions is replicated to 2 wo partitions via a stride-0 dim."""
    ap = xrb_d[:, lo:hi]
    if XB:
        ap = ap.rearrange("q (r c) -> q r c", r=1).to_broadcast([W, 2, hi - lo])
    return ap


def _build_nc():
    # Cross-engine ordering is fully explicit via semaphores below; the
    # remaining same-engine WAW (DVE scratch ring) is safe on HW because DVE
    # drains its pipe between ops, so skip the detector's extra waits.
    nc = bass.Bass(detect_race_conditions=False)
    if WOMAJOR:
        mw_d = nc.declare_dram_parameter("mw", [WO, HO_PC * QF], DT, isOutput=False)
    else:
        mw_d = nc.declare_dram_parameter("mw", [HO_PC, WO, QF], DT, isOutput=False)
    xrb_d = nc.declare_dram_parameter(
        "xrb", [W if XB else WO, NROWS * RW], DT, isOutput=False
    )
    out_d = nc.declare_dram_parameter("out", [WO, HO_PC * F], ADT, isOutput=True)

    FV = F - GF  # filters on DVE; GpSimd takes the last GF filters

    with ExitStack() as ctx:
        xrow = ctx.enter_context(nc.sbuf_tensor([WO, NROWS * RW], DT))
        mwbuf = ctx.enter_context(nc.sbuf_tensor([WO, NBUF * MAXR * QF], DT))
        scr_v = ctx.enter_context(nc.sbuf_tensor([WO, NSCR * K], DT))
        out_sb = ctx.enter_context(nc.sbuf_tensor([WO, HO_PC * F], ADT))
        slot_sem = [ctx.enter_context(nc.semaphore(f"slot{j}")) for j in range(NBUF)]
        misc_sem = ctx.enter_context(nc.semaphore("misc"))
        cmp_v = ctx.enter_context(nc.semaphore("cmp_v"))   # DVE fused ops done
        if GF:
            scr_g = ctx.enter_context(nc.sbuf_tensor([WO, NSCR * K], DT))
            cmp_g = ctx.enter_context(nc.semaphore("cmp_g"))
        if TT2:
            prodbuf = ctx.enter_context(
                nc.sbuf_tensor([WO, NPROD * PAIR * K], DT)
            )
            cmp_r = ctx.enter_context(nc.semaphore("cmp_r"))  # reduces done
        if ACTR:
            prodbuf = ctx.enter_context(
                nc.sbuf_tensor([WO, NPROD * MAXB * K], DT)
            )
            trash_a = ctx.enter_context(nc.sbuf_tensor([WO, K], DT))
            cmp_s = ctx.enter_context(nc.semaphore("cmp_s"))  # DVE STTs done
            cmp_a = ctx.enter_context(nc.semaphore("cmp_a"))  # ACT reduces done
            if GM:
                cmp_gm = ctx.enter_context(nc.semaphore("cmp_gm"))  # GPS mults
            if DF:
                cmp_f = ctx.enter_context(nc.semaphore("cmp_f"))  # folds done
        if F1:
            prodbuf = ctx.enter_context(
                nc.sbuf_tensor([WO, NPROD * PAIR * K], DT)
            )
            halfbuf = ctx.enter_context(
                nc.sbuf_tensor([WO, NPROD * PAIR * (K // 2)], DT)
            )
            trash_a = ctx.enter_context(nc.sbuf_tensor([WO, K], DT))
            cmp_a = ctx.enter_context(nc.semaphore("cmp_a"))  # ACT reduces done
        if HF:
            hf_sem = ctx.enter_context(nc.semaphore("hf"))
        if XTAIL:
            xt_sem = ctx.enter_context(nc.semaphore("xt"))
        block = ctx.enter_context(nc.Block())

        def slot_ap(j, rows):
            base = j * MAXR * QF
            return mwbuf[:, base : base + rows * QF]

        @block.sync
        def _(sync):
            for i in range(NT):
                j = i % NBUF
                rows, row0 = SCHED[i], ROW0[i]
                if i >= NBUF:
                    # both engines finished reading the slot's previous tile
                    prev = i - NBUF
                    done = ROW0[prev] + SCHED[prev]
                    if TT2 or F1:
                        sync.wait_ge(cmp_v, done // 2)
                    elif ACTR:
                        sync.wait_ge(cmp_s, CUM_S[done // 2])
                        if GM:
                            sync.wait_ge(cmp_gm, done // 2)
                    else:
                        if FV:
                            sync.wait_ge(cmp_v, done * FV)
                        if GF:
                            sync.wait_ge(cmp_g, done * GF)
                if WOMAJOR:
                    if HF and i == 0:
                        # rows 0-1 come from the ACT ring (head-feed)
                        sync.dma_start(
                            out=slot_ap(0, rows)[:, 2 * QF : rows * QF],
                            in_=mw_d[:, 2 * QF : rows * QF],
                        ).then_inc(slot_sem[0], 16)
                    else:
                        sync.dma_start(
                            out=slot_ap(j, rows),
                            in_=mw_d[:, row0 * QF : (row0 + rows) * QF],
                        ).then_inc(slot_sem[j], 16)
                else:
                    sync.dma_start(
                        out=slot_ap(j, rows).rearrange("p (h q) -> p h q", h=rows),
                        in_=mw_d[row0 : row0 + rows].rearrange("h w q -> w h q"),
                    ).then_inc(slot_sem[j], 16)
                if XTAIL and i == XTI:
                    sync.dma_start(
                        out=xrow[:, XH * RW :],
                        in_=_xrb_src(xrb_d, XH * RW, NROWS * RW),
                    ).then_inc(xt_sem, 16)
            # overlap the bulk of the output store with the tail tiles;
            # only a 12KB final piece remains after the last compute op
            if TT2:
                sync.wait_ge(cmp_r, 30)
            elif F1:
                sync.wait_ge(cmp_a, 30 * PAIR)
            elif ACTR:
                sync.wait_ge(cmp_s, CUM_S[P1])
                sync.wait_ge(cmp_a, CUM_A[P1])
            else:
                if FV:
                    sync.wait_ge(cmp_v, 60 * FV)
                if GF:
                    sync.wait_ge(cmp_g, 60 * GF)
            c1 = (2 * P1 if ACTR else 60) * F
            sync.dma_start(
                out=out_d[:, :c1], in_=out_sb[:, :c1]
            ).then_inc(misc_sem, 16)
            if TT2:
                sync.wait_ge(cmp_r, HO_PC // 2)
            elif F1:
                sync.wait_ge(cmp_a, (HO_PC // 2) * PAIR)
            elif ACTR:
                sync.wait_ge(cmp_s, CUM_S[NPAIRS])
                sync.wait_ge(cmp_a, CUM_A[NPAIRS])
            else:
                if FV:
                    sync.wait_ge(cmp_v, HO_PC * FV)
                if GF:
                    sync.wait_ge(cmp_g, HO_PC * GF)
            sync.dma_start(
                out=out_d[:, c1:], in_=out_sb[:, c1:]
            ).then_inc(misc_sem, 16)

        @block.scalar
        def _(scalar):
            # xrb on the ACT HWDGE ring so it doesn't head-of-line block the
            # meta_w stream on the SP ring; head chunk first so DVE can start
            # tile 0 after ~480KB instead of 3.3MB.
            scalar.dma_start(
                out=xrow[:, : XH * RW], in_=_xrb_src(xrb_d, 0, XH * RW)
            ).then_inc(misc_sem, 16)
            if HF:
                scalar.dma_start(
                    out=slot_ap(0, SCHED[0])[:, : 2 * QF],
                    in_=mw_d[:, : 2 * QF],
                ).then_inc(hf_sem, 16)
            if not XTAIL:
                scalar.dma_start(
                    out=xrow[:, XH * RW :],
                    in_=_xrb_src(xrb_d, XH * RW, NROWS * RW),
                ).then_inc(misc_sem, 16)
            if F1:
                KH = K // 2
                for s in range(HO_PC // 2):
                    scalar.wait_ge(cmp_v, s + 1)  # pair's folded halves ready
                    half = halfbuf[
                        :, (s % NPROD) * PAIR * KH : (s % NPROD + 1) * PAIR * KH
                    ]
                    for g in range(PAIR):
                        scalar.activation(
                            out=trash_a[:, :KH],
                            in_=half[:, g * KH : (g + 1) * KH],
                            func=mybir.ActivationFunctionType.Copy,
                            accum_out=out_sb[:, s * PAIR + g : s * PAIR + g + 1],
                        ).then_inc(cmp_a, 1)
            if ACTR:
                for s in range(HO_PC // 2):
                    scalar.wait_ge(cmp_v, s + 1)  # pair's products ready
                    if GM:
                        scalar.wait_ge(cmp_gm, s + 1)
                    if DF:
                        scalar.wait_ge(cmp_f, 16 * (s + 1))  # fold landed
                    b = B_OF[s]
                    prod = prodbuf[
                        :, (s % NPROD) * MAXB * K : (s % NPROD) * MAXB * K + b * K
                    ]
                    red_w = K // 2 if DF else K  # folded segments are half-width
                    for g in range(b):
                        acc_col = s * PAIR + (PAIR - b) + g
                        scalar.activation(
                            out=trash_a[:, :red_w],
                            in_=prod[:, g * K : g * K + red_w],
                            func=mybir.ActivationFunctionType.Copy,
                            accum_out=out_sb[:, acc_col : acc_col + 1],
                        ).then_inc(cmp_a, 1)

        def compute_body(eng, f_lo, f_hi, scr, cmp_sem):
            eng.wait_ge(misc_sem, 16)
            nv = 0
            xrow_full_waited = False
            for i in range(NT):
                j, p = i % NBUF, i // NBUF
                rows = SCHED[i]
                if not xrow_full_waited and (ROW0[i] + rows - 1) // 2 + 2 >= XH:
                    eng.wait_ge(xt_sem if XTAIL else misc_sem, 16 if XTAIL else 32)  # rest of xrow
                    xrow_full_waited = True
                eng.wait_ge(slot_sem[j], 16 * (p + 1))
                if FMAJOR:
                    mw4 = slot_ap(j, rows).rearrange(
                        "p (h f k) -> p h f k", h=rows, f=F
                    )
                else:
                    mw4 = slot_ap(j, rows).rearrange(
                        "p (h k f) -> p h k f", h=rows, f=F
                    )
                for r in range(rows):
                    ho = ROW0[i] + r
                    win = xrow[:, (ho // 2) * RW : (ho // 2) * RW + KS * RW]
                    for f in range(f_lo, f_hi):
                        if SELF_WAITS and nv >= NSCR:
                            eng.wait_ge(cmp_sem, nv - NSCR + 1)
                        in0 = mw4[:, r, f, :] if FMAJOR else mw4[:, r, :, f]
                        o = scr[:, (nv % NSCR) * K : (nv % NSCR + 1) * K]
                        acc = out_sb[:, ho * F + f : ho * F + f + 1]
                        if TTR and eng is nc.vector:
                            eng.tensor_tensor_reduce(
                                out=o,
                                in0=in0,
                                in1=win,
                                scale=1.0,
                                scalar=0.0,
                                op0=mybir.AluOpType.mult,
                                op1=mybir.AluOpType.add,
                                accum_out=acc,
                            ).then_inc(cmp_sem, 1)
                        else:
                            eng.scalar_tensor_tensor(
                                out=o,
                                in0=in0,
                                scalar=1.0,
                                in1=win,
                                op0=mybir.AluOpType.mult,
                                op1=mybir.AluOpType.mult,
                                accum_out=acc,
                            ).then_inc(cmp_sem, 1)
                        nv += 1

        def tt2_body(vector):
            vector.wait_ge(misc_sem, 16)
            ns = 0  # pair index
            xrow_full_waited = False
            for i in range(NT):
                j, p = i % NBUF, i // NBUF
                rows = SCHED[i]
                if not xrow_full_waited and (ROW0[i] + rows - 1) // 2 + 2 >= XH:
                    vector.wait_ge(xt_sem if XTAIL else misc_sem, 16 if XTAIL else 32)  # rest of xrow
                    xrow_full_waited = True
                vector.wait_ge(slot_sem[j], 16 * (p + 1))
                sl = slot_ap(j, rows)
                for sp in range(rows // 2):
                    s = ROW0[i] // 2 + sp  # global pair index == hs
                    win = xrow[:, s * RW : s * RW + KS * RW]
                    win_b = win.rearrange("p (r k) -> p r k", r=1).to_broadcast(
                        [WO, PAIR, K]
                    )
                    in0 = sl[:, 2 * sp * QF : (2 * sp + 2) * QF].rearrange(
                        "p (g k) -> p g k", g=PAIR
                    )
                    prod = prodbuf[
                        :, (ns % NPROD) * PAIR * K : (ns % NPROD + 1) * PAIR * K
                    ].rearrange("p (g k) -> p g k", g=PAIR)
                    vector.tensor_tensor(
                        out=prod, in0=in0, in1=win_b, op=mybir.AluOpType.mult
                    ).then_inc(cmp_v, 1)
                    vector.tensor_reduce(
                        out=out_sb[:, s * PAIR : (s + 1) * PAIR],
                        in_=prod,
                        axis=mybir.AxisListType.X,
                        op=mybir.AluOpType.add,
                    ).then_inc(cmp_r, 1)
                    ns += 1

        def actr_body(vector):
            vector.wait_ge(misc_sem, 16)
            ns = 0  # pair index
            nv = 0  # STT scratch ring index
            xrow_full_waited = False
            for i in range(NT):
                j, p = i % NBUF, i // NBUF
                rows = SCHED[i]
                if not xrow_full_waited and (ROW0[i] + rows - 1) // 2 + 2 >= XH:
                    vector.wait_ge(xt_sem if XTAIL else misc_sem, 16 if XTAIL else 32)  # rest of xrow
                    xrow_full_waited = True
                if not (HF and i == 0):
                    vector.wait_ge(slot_sem[j], 16 * (p + 1))
                sl = slot_ap(j, rows)
                for sp in range(rows // 2):
                    s = ROW0[i] // 2 + sp  # global pair index == hs
                    if HF and i == 0:
                        # pair 0 from the ACT-ring half, pair 1 from sync's
                        vector.wait_ge(hf_sem if sp == 0 else slot_sem[0], 16)
                    win = xrow[:, s * RW : s * RW + KS * RW]
                    base = 2 * sp * QF
                    # packed mult for the last b segments -> prod ring
                    b = B_OF[s]
                    if s >= NPROD:
                        vector.wait_ge(cmp_a, CUM_A[s - NPROD + 1])
                    nseg = b - GM
                    win_b = win.rearrange("p (r k) -> p r k", r=1).to_broadcast(
                        [WO, nseg, K]
                    )
                    prod = prodbuf[
                        :,
                        (ns % NPROD) * MAXB * K : (ns % NPROD) * MAXB * K
                        + nseg * K,
                    ].rearrange("p (g k) -> p g k", g=nseg)
                    vector.tensor_tensor(
                        out=prod,
                        in0=sl[
                            :, base + (PAIR - b) * K : base + (PAIR - GM) * K
                        ].rearrange("p (g k) -> p g k", g=nseg),
                        in1=win_b,
                        op=mybir.AluOpType.mult,
                    ).then_inc(cmp_v, 1)
                    # fused STT for the first PAIR-b segments
                    for g in range(PAIR - b):
                        vector.scalar_tensor_tensor(
                            out=scr_v[:, (nv % NSCR) * K : (nv % NSCR + 1) * K],
                            in0=sl[:, base + g * K : base + (g + 1) * K],
                            scalar=1.0,
                            in1=win,
                            op0=mybir.AluOpType.mult,
                            op1=mybir.AluOpType.mult,
                            accum_out=out_sb[:, s * PAIR + g : s * PAIR + g + 1],
                        ).then_inc(cmp_s, 1)
                        nv += 1
                    ns += 1

        def gm_body(g):
            g.wait_ge(misc_sem, 16)
            xrow_full_waited = False
            for i in range(NT):
                j, p = i % NBUF, i // NBUF
                rows = SCHED[i]
                if not xrow_full_waited and (ROW0[i] + rows - 1) // 2 + 2 >= XH:
                    g.wait_ge(xt_sem if XTAIL else misc_sem, 16 if XTAIL else 32)
                    xrow_full_waited = True
                g.wait_ge(slot_sem[j], 16 * (p + 1))
                sl = slot_ap(j, rows)
                for sp in range(rows // 2):
                    s = ROW0[i] // 2 + sp
                    win = xrow[:, s * RW : s * RW + KS * RW]
                    base = 2 * sp * QF
                    if s >= NPROD:
                        g.wait_ge(cmp_a, (s - NPROD + 1) * ACTR)
                    win_b = win.rearrange("p (r k) -> p r k", r=1).to_broadcast(
                        [WO, GM, K]
                    )
                    prod = prodbuf[
                        :,
                        (s % NPROD) * ACTR * K
                        + (ACTR - GM) * K : (s % NPROD) * ACTR * K
                        + ACTR * K,
                    ].rearrange("p (g k) -> p g k", g=GM)
                    g.tensor_tensor(
                        out=prod,
                        in0=sl[
                            :, base + (PAIR - GM) * K : base + PAIR * K
                        ].rearrange("p (g k) -> p g k", g=GM),
                        in1=win_b,
                        op=mybir.AluOpType.mult,
                    ).then_inc(cmp_gm, 1)

        def f1_body(vector):
            KH = K // 2
            vector.wait_ge(misc_sem, 16)
            xrow_full_waited = False
            for i in range(NT):
                j, p = i % NBUF, i // NBUF
                rows = SCHED[i]
                if not xrow_full_waited and (ROW0[i] + rows - 1) // 2 + 2 >= XH:
                    vector.wait_ge(xt_sem if XTAIL else misc_sem, 16 if XTAIL else 32)  # rest of xrow
                    xrow_full_waited = True
                vector.wait_ge(slot_sem[j], 16 * (p + 1))
                sl = slot_ap(j, rows)
                for sp in range(rows // 2):
                    s = ROW0[i] // 2 + sp  # global pair index == hs
                    win = xrow[:, s * RW : s * RW + KS * RW]
                    if s >= NPROD:
                        vector.wait_ge(cmp_a, (s - NPROD + 1) * PAIR)
                    win_b = win.rearrange("p (r k) -> p r k", r=1).to_broadcast(
                        [WO, PAIR, K]
                    )
                    prod = prodbuf[
                        :, (s % NPROD) * PAIR * K : (s % NPROD + 1) * PAIR * K
                    ].rearrange("p (g k) -> p g k", g=PAIR)
                    vector.tensor_tensor(
                        out=prod,
                        in0=sl[:, 2 * sp * QF : (2 * sp + 2) * QF].rearrange(
                            "p (g k) -> p g k", g=PAIR
                        ),
                        in1=win_b,
                        op=mybir.AluOpType.mult,
                    )
                    half = halfbuf[
                        :,
                        (s % NPROD) * PAIR * KH : (s % NPROD + 1) * PAIR * KH,
                    ].rearrange("p (g k) -> p g k", g=PAIR)
                    vector.tensor_tensor(
                        out=half,
                        in0=prod[:, :, :KH],
                        in1=prod[:, :, KH:],
                        op=mybir.AluOpType.add,
                    ).then_inc(cmp_v, 1)

        if TT2:

            @block.vector
            def _(vector):
                tt2_body(vector)

        elif F1:

            @block.vector
            def _(vector):
                f1_body(vector)

        elif ACTR:

            @block.vector
            def _(vector):
                actr_body(vector)

            if GM:

                @block.gpsimd
                def _(g):
                    gm_body(g)

            if DF:

                @block.gpsimd
                def _(g):
                    for s in range(HO_PC // 2):
                        g.wait_ge(cmp_v, s + 1)  # pair's products written
                        prod = prodbuf[
                            :,
                            (s % NPROD) * ACTR * K : (s % NPROD + 1) * ACTR * K,
                        ].rearrange("p (g k) -> p g k", g=ACTR)
                        g.dma_start(
                            out=prod[:, :, : K // 2],
                            in_=prod[:, :, K // 2 :],
                            accum_op=mybir.AluOpType.add,
                        ).then_inc(cmp_f, 16)

        elif FV:

            @block.vector
            def _(vector):
                compute_body(vector, 0, FV, scr_v, cmp_v)

        if GF and not TT2:

            @block.gpsimd
            def _(gpsimd):
                compute_body(gpsimd, FV, F, scr_g, cmp_g)

    return nc


def _prep_xrb(x):
    """Per-core duplicated patch-row tensors.

    xrb[ci][wo, hpl*RW + dk1*C + c] = x_pad[b, hs0+hpl, wo//2 + dk1, c]
    where x_pad has 1 zero row/col of padding on each side.
    """
    from numpy.lib.stride_tricks import sliding_window_view

    out = []
    for ci in range(N_CORES):
        b, hs0 = ci // CORES_PER_B, (ci % CORES_PER_B) * NHS
        xp = np.pad(x[b], ((1, 1), (1, 1), (0, 0)))          # [66, 66, 64]
        rows = xp[hs0 : hs0 + NROWS]                          # [34, 66, 64]
        win = sliding_window_view(rows, KS, axis=1)           # [34, 64(ws), 64(c), 3(dk1)]
        win = win.transpose(0, 1, 3, 2).reshape(NROWS, W, RW)  # [34, 64, 192]
        if XB:
            out.append(
                np.ascontiguousarray(win.transpose(1, 0, 2))
                .reshape(W, NROWS * RW)
                .astype(NPDT)
            )
        else:
            dup = np.repeat(win, 2, axis=1)                   # [34, 128, 192]
            out.append(
                np.ascontiguousarray(dup.transpose(1, 0, 2))
                .reshape(WO, NROWS * RW)
                .astype(NPDT)
            )
    return out


def _ensure_axon_hooks_module():
    """This image's antenv lacks axon_hooks; run_bass_kernel_spmd imports it
    when BASS_TRACE is set. Provide it (registering the real NTFF hook when
    available) so tracing degrades gracefully instead of crashing."""
    try:
        import antenv.axon_hooks  # noqa: F401
        return
    except ImportError:
        pass
    import sys
    import types

    try:
        import antenv
    except ImportError:
        return
    mod = types.ModuleType("antenv.axon_hooks")
    _hook = [None]
    mod.set_axon_ntff_profile_hook = lambda h: _hook.__setitem__(0, h)
    mod.get_axon_ntff_profile_hook = lambda: _hook[0]
    sys.modules["antenv.axon_hooks"] = mod
    antenv.axon_hooks = mod
    try:
        from trn_agent_boot.trn_boot import _ntff_profile_via_ctypes

        h = _ntff_profile_via_ctypes("/opt/axon/libaxon_pjrt.so")
        if h is not None:
            _hook[0] = h
    except Exception:
        pass


_ensure_axon_hooks_module()

last_results = None  # BassKernelResults of the most recent kernel() call


def kernel(x, meta_w):
    global _CACHED, last_results
    x = np.ascontiguousarray(np.asarray(x, dtype=np.float32))
    meta_w = np.asarray(meta_w, dtype=np.float32)

    if _CACHED is None:
        _CACHED = _build_nc()
    nc = _CACHED

    xrbs = _prep_xrb(x)
    in_maps = []
    for ci in range(N_CORES):
        b, ho0 = ci // CORES_PER_B, (ci % CORES_PER_B) * HO_PC
        mw_c = meta_w[b, ho0 : ho0 + HO_PC]
        if WOMAJOR:
            # [HO_PC, WO, K, F] -> [WO, HO_PC, F, K] (F-major within a row)
            mw_c = (
                mw_c.reshape(HO_PC, WO, K, F)
                .transpose(1, 0, 3, 2)
                .astype(NPDT)
                .reshape(WO, HO_PC * QF)
            )
        elif FMAJOR:
            mw_c = (
                mw_c.reshape(HO_PC, WO, K, F)
                .transpose(0, 1, 3, 2)
                .astype(NPDT)
                .reshape(HO_PC, WO, QF)
            )
        else:
            mw_c = mw_c.astype(NPDT)
        in_maps.append({"mw": mw_c, "xrb": xrbs[ci]})

    res = run_bass_kernel_spmd(nc, in_maps, list(range(N_CORES)))
    last_results = res

    out = np.empty((B, HO, WO, F), np.float32)
    for ci in range(N_CORES):
        b, ho0 = ci // CORES_PER_B, (ci % CORES_PER_B) * HO_PC
        o = res.results[ci]["out"].astype(np.float32).reshape(WO, HO_PC, F)
        out[b, ho0 : ho0 + HO_PC] = o.transpose(1, 0, 2)
    return out

